# revision 8
# baseline (speedup 1.0000x reference)
"""Multi-head attention (B=8, S=1024, H=768, NH=12) on 8 Trainium2 cores.

Strategy: pure data parallelism — core c computes batch element c end-to-end.
Weights are passed host-pre-transposed ([in,out] layout) to every core; all
matmuls run in float32r (TF32-like, full PE rate at N>=256, ~1.5e-4 rel err).

Per-core dataflow (batch b, S=1024 tokens, H=768 channels, 12 heads x 64):
  1. PE-transpose Q,K,V tiles -> QT/KT/VT [h, s] layouts in SBUF (f32r).
  2. qT = WqT.T @ QT, kT = WkT.T @ KT   ([o, s] layout; head-pair per 128-row
     tile), v = VT.T @ WvT ([s, o] natural) written into v_aug with a ones
     column appended per head (-> softmax denominator comes out of the AV
     matmul for free).
  3. Per head-pair j, per 512-wide query block: scoresT[sk, sq] via row-tiled
     concurrent K=64 matmul pairs; exp on ScalarE (scale=1/8 folded in, no
     max-subtraction needed -- |scores| < ~4 by construction); AV matmul with
     lhsT = v_aug[sk, 65] accumulated over sk-tiles -> psum rows 0:64 = c.T
     unnormalized, row 64 = softmax denominator l.
  4. recip(l) on DVE, broadcast via K=1 matmul, normalize in the PSUM->SBUF
     copy (DVE multiply) -> cT [h, s] layout.
  5. out = cT.T @ WoT per s-tile -> DMA to DRAM.
"""

import numpy as np

import concourse.bass as bass
import concourse.mybir as mybir
import concourse.tile as tile
from concourse import bacc
from concourse import bass_utils
from concourse.masks import make_identity

dt = mybir.dt
AF = mybir.ActivationFunctionType

B, S, H, NH = 8, 1024, 768, 12
DK = H // NH            # 64
N_CORES = 8
HT = H // 128           # 6 h-tiles
ST = S // 128           # 8 s-tiles
PAIRS = NH // 2         # 6 head pairs
SQB = S // 512          # 2 query blocks of 512
VA = NH * (DK + 1)      # 780: v_aug row width per s-tile (12 heads x 65)
SCALE = 1.0 / float(np.sqrt(np.float32(DK)))

_NC_CACHE = {}


def _build_nc():
    if "nc" in _NC_CACHE:
        return _NC_CACHE["nc"]
    nc = bacc.Bacc("TRN2", target_bir_lowering=False, debug=False,
                   num_devices=N_CORES)
    Q = nc.dram_tensor("Qb", [S, H], dt.float32, kind="ExternalInput").ap()
    K = nc.dram_tensor("Kb", [S, H], dt.float32, kind="ExternalInput").ap()
    V = nc.dram_tensor("Vb", [S, H], dt.float32, kind="ExternalInput").ap()
    WqT = nc.dram_tensor("WqT", [H, H], dt.float32, kind="ExternalInput").ap()
    WkT = nc.dram_tensor("WkT", [H, H], dt.float32, kind="ExternalInput").ap()
    WvT = nc.dram_tensor("WvT", [H, H], dt.float32, kind="ExternalInput").ap()
    WoT = nc.dram_tensor("WoT", [H, H], dt.float32, kind="ExternalInput").ap()
    Ob = nc.dram_tensor("Ob", [S, H], dt.float32, kind="ExternalOutput").ap()

    with tile.TileContext(nc) as tc:
        _emit(nc, tc, Q, K, V, WqT, WkT, WvT, WoT, Ob)
    nc.finalize()
    _NC_CACHE["nc"] = nc
    return nc


def _emit(nc, tc, Q, K, V, WqT, WkT, WvT, WoT, Ob):
    with (
        tc.tile_pool(name="const", bufs=1) as constp,
        tc.tile_pool(name="wstage", bufs=2) as wstage,
        tc.tile_pool(name="wr", bufs=2) as wr,
        tc.tile_pool(name="natstage", bufs=3) as natstage,
        tc.tile_pool(name="xtraw", bufs=1) as xtraw,
        tc.tile_pool(name="proj", bufs=1) as projp,
        tc.tile_pool(name="wtil", bufs=3) as wtil,
        tc.tile_pool(name="small", bufs=4) as smallp,
        tc.tile_pool(name="bcast", bufs=2) as bcastp,
        tc.tile_pool(name="outstage", bufs=2) as outstage,
    ):
        ident = constp.tile([128, 128], dt.float32, tag="ident")
        make_identity(nc, ident[:])

        qt = projp.tile([128, PAIRS * S], dt.float32r, tag="qt")
        kt = projp.tile([128, PAIRS * S], dt.float32r, tag="kt")
        v_aug = projp.tile([128, ST * VA], dt.float32r, tag="vaug")
        cT = projp.tile([128, PAIRS * S], dt.float32r, tag="ct")

        # ================= prep phase: transposes + projections ============
        with (
            tc.tile_pool(name="ps_ta", bufs=2, space="PSUM") as ps_ta,
            tc.tile_pool(name="ps_tb", bufs=2, space="PSUM") as ps_tb,
            tc.tile_pool(name="ps_p", bufs=3, space="PSUM") as ps_p,
        ):
            def load_weight(wdram):
                w = wr.tile([128, HT * H], dt.float32r, tag="w")
                for ht in range(HT):
                    stg = wstage.tile([128, H], dt.float32, tag="wstg")
                    nc.sync.dma_start(stg[:], wdram[ht * 128:(ht + 1) * 128, :])
                    nc.vector.tensor_copy(w[:, ht * H:(ht + 1) * H], stg[:])
                return w

            def transpose_in(xdram, dest):
                # dest: [128, HT*S] f32r ; dest[p, ht*S + s] = X[s, ht*128+p]
                destr = dest.rearrange("p (ht s) -> p ht s", ht=HT)
                for st in range(ST):
                    nat = natstage.tile([128, H], dt.float32, tag="nat")
                    nc.sync.dma_start(nat[:], xdram[st * 128:(st + 1) * 128, :])
                    pa = ps_ta.tile([128, 512], dt.float32, tag="ps_ta")
                    pb = ps_tb.tile([128, 256], dt.float32, tag="ps_tb")
                    for ht in range(HT):
                        pdst = (pa[:, ht * 128:(ht + 1) * 128] if ht < 4
                                else pb[:, (ht - 4) * 128:(ht - 3) * 128])
                        nc.tensor.transpose(
                            pdst, nat[:, ht * 128:(ht + 1) * 128], ident[:])
                    ss = slice(st * 128, (st + 1) * 128)
                    nc.vector.tensor_copy(
                        destr[:, 0:4, ss],
                        pa[:].rearrange("p (ht s) -> p ht s", ht=4))
                    nc.vector.tensor_copy(
                        destr[:, 4:6, ss],
                        pb[:].rearrange("p (ht s) -> p ht s", ht=2))

            def proj_os(w, xt, dest):
                # dest [128, PAIRS*S] f32r; dest[p, j*S+s] head-pair tile j
                for j in range(HT):
                    for b in range(S // 512):
                        ps = ps_p.tile([128, 512], dt.float32, tag="ps_p")
                        for ht in range(HT):
                            nc.tensor.matmul(
                                ps[:],
                                w[:, ht * H + j * 128: ht * H + (j + 1) * 128],
                                xt[:, ht * S + b * 512: ht * S + (b + 1) * 512],
                                start=(ht == 0), stop=(ht == HT - 1))
                        nc.scalar.copy(
                            dest[:, j * S + b * 512: j * S + (b + 1) * 512],
                            ps[:])

            def proj_v(w, vt, dest):
                for st in range(ST):
                    row = dest[:, st * VA:(st + 1) * VA]
                    rowr = row.rearrange("p (n d) -> p n d", d=DK + 1)
                    for o0, ow in ((0, 512), (512, 256)):
                        ps = ps_p.tile([128, 512], dt.float32, tag="ps_p")
                        for ht in range(HT):
                            nc.tensor.matmul(
                                ps[:, 0:ow],
                                vt[:, ht * S + st * 128: ht * S + (st + 1) * 128],
                                w[:, ht * H + o0: ht * H + o0 + ow],
                                start=(ht == 0), stop=(ht == HT - 1))
                        psr = ps[:, 0:ow].rearrange("p (n d) -> p n d", d=DK)
                        n0 = o0 // DK
                        nw = ow // DK
                        nc.vector.tensor_copy(rowr[:, n0:n0 + nw, 0:DK],
                                              psr[:])
                    nc.scalar.activation(
                        rowr[:, :, DK:DK + 1],
                        ident[:, 0:NH].rearrange("p (n o) -> p n o", o=1),
                        AF.Identity, bias=1.0, scale=0.0)

            qt_raw = xtraw.tile([128, HT * S], dt.float32r, tag="xt")
            transpose_in(Q, qt_raw)
            wq = load_weight(WqT)
            proj_os(wq, qt_raw, qt)

            kt_raw = xtraw.tile([128, HT * S], dt.float32r, tag="xt")
            transpose_in(K, kt_raw)
            wk = load_weight(WkT)
            proj_os(wk, kt_raw, kt)

            vt_raw = xtraw.tile([128, HT * S], dt.float32r, tag="xt")
            transpose_in(V, vt_raw)
            wv = load_weight(WvT)
            proj_v(wv, vt_raw, v_aug)

            wo = load_weight(WoT)

        # ================= attention + output projection ===================
        var = v_aug[:, :].rearrange("p (st n d) -> p st n d", st=ST, d=DK + 1)
        with (
            tc.tile_pool(name="ps_s", bufs=4, space="PSUM") as ps_s,
            tc.tile_pool(name="ps_c", bufs=2, space="PSUM") as ps_c,
            tc.tile_pool(name="ps_o", bufs=2, space="PSUM") as ps_o,
        ):
            for j in range(PAIRS):
                for b in range(SQB):
                    sq = slice(j * S + b * 512, j * S + (b + 1) * 512)
                    pc = []
                    for h in range(2):
                        pch = ps_c.tile([DK + 1, 512], dt.float32, tag="ps_c")
                        pc.append(pch)
                    for t in range(ST):
                        sk = slice(j * S + t * 128, j * S + (t + 1) * 128)
                        for h in range(2):
                            p0 = h * DK
                            pss = ps_s.tile([128, 512], dt.float32, tag="ps_s")
                            nc.tensor.matmul(
                                pss[:],
                                kt[p0:p0 + DK, sk], qt[p0:p0 + DK, sq],
                                start=True, stop=True, tile_position=(p0, 0))
                            w_t = wtil.tile([128, 512], dt.float32r, tag="wt")
                            nc.scalar.activation(w_t[:], pss[:], AF.Exp,
                                                 scale=SCALE)
                            nc.tensor.matmul(
                                pc[h][:], var[:, t, 2 * j + h, :], w_t[:],
                                start=(t == 0), stop=(t == ST - 1),
                                skip_group_check=True)
                    for h in range(2):
                        rec = smallp.tile([1, 512], dt.float32, tag="rec")
                        nc.vector.reciprocal(rec[:], pc[h][DK:DK + 1, :])
                        bc = bcastp.tile([DK, 512], dt.float32, tag="bc")
                        nc.gpsimd.partition_broadcast(bc[:], rec[:])
                        nc.vector.tensor_mul(
                            cT[h * DK:(h + 1) * DK, sq],
                            pc[h][0:DK, :], bc[:])

            for st in range(ST):
                out_sb = outstage.tile([128, H], dt.float32, tag="out")
                for o0, ow in ((0, 512), (512, 256)):
                    ps = ps_o.tile([128, 512], dt.float32, tag="ps_o")
                    for j in range(PAIRS):
                        nc.tensor.matmul(
                            ps[:, 0:ow],
                            cT[:, j * S + st * 128: j * S + (st + 1) * 128],
                            wo[:, j * H + o0: j * H + o0 + ow],
                            start=(j == 0), stop=(j == PAIRS - 1))
                    nc.scalar.copy(out_sb[:, o0:o0 + ow], ps[:, 0:ow])
                nc.sync.dma_start(Ob[st * 128:(st + 1) * 128, :], out_sb[:])


def kernel(Q, K, V, Wq, Wk, Wv, Wo):
    nc = _build_nc()
    wqt = np.ascontiguousarray(np.asarray(Wq).T)
    wkt = np.ascontiguousarray(np.asarray(Wk).T)
    wvt = np.ascontiguousarray(np.asarray(Wv).T)
    wot = np.ascontiguousarray(np.asarray(Wo).T)
    in_maps = []
    for c in range(N_CORES):
        in_maps.append({
            "Qb": np.ascontiguousarray(np.asarray(Q)[c]),
            "Kb": np.ascontiguousarray(np.asarray(K)[c]),
            "Vb": np.ascontiguousarray(np.asarray(V)[c]),
            "WqT": wqt, "WkT": wkt, "WvT": wvt, "WoT": wot,
        })
    res = bass_utils.run_bass_kernel_spmd(nc, in_maps,
                                          core_ids=list(range(N_CORES)))
    out = np.stack([res.results[c]["Ob"] for c in range(N_CORES)], axis=0)
    return out.astype(np.float32)


# revision 10
# speedup vs baseline: 99.2202x; 99.2202x over previous
"""Multi-head attention (B=8, S=1024, H=768, NH=12) on 8 Trainium2 cores.

Strategy: pure data parallelism — core c computes batch element c end-to-end.
Weights are passed host-pre-transposed ([in,out] layout) to every core; all
matmuls run in float32r (TF32-like, full PE rate at N>=256, ~1.5e-4 rel err).

Per-core dataflow (batch b, S=1024 tokens, H=768 channels, 12 heads x 64):
  1. PE-transpose Q,K,V tiles -> QT/KT/VT [h, s] layouts in SBUF (f32r).
  2. qT = WqT.T @ QT, kT = WkT.T @ KT   ([o, s] layout; head-pair per 128-row
     tile), v = VT.T @ WvT ([s, o] natural) written into v_aug with a ones
     column appended per head (-> softmax denominator comes out of the AV
     matmul for free).
  3. Per head-pair j, per 512-wide query block: scoresT[sk, sq] via row-tiled
     concurrent K=64 matmul pairs; exp on ScalarE (scale=1/8 folded in, no
     max-subtraction needed -- |scores| < ~4 by construction); AV matmul with
     lhsT = v_aug[sk, 65] accumulated over sk-tiles -> psum rows 0:64 = c.T
     unnormalized, row 64 = softmax denominator l.
  4. recip(l) on DVE, broadcast via K=1 matmul, normalize in the PSUM->SBUF
     copy (DVE multiply) -> cT [h, s] layout.
  5. out = cT.T @ WoT per s-tile -> DMA to DRAM.
"""

import numpy as np

import concourse.bass as bass
import concourse.mybir as mybir
import concourse.tile as tile
from concourse import bacc
from concourse import bass_utils
from concourse.masks import make_identity

dt = mybir.dt
AF = mybir.ActivationFunctionType

B, S, H, NH = 8, 1024, 768, 12
DK = H // NH            # 64
N_CORES = 8
HT = H // 128           # 6 h-tiles
ST = S // 128           # 8 s-tiles
PAIRS = NH // 2         # 6 head pairs
SQB = S // 512          # 2 query blocks of 512
VA = NH * (DK + 1)      # 780: v_aug row width per s-tile (12 heads x 65)
SCALE = 1.0 / float(np.sqrt(np.float32(DK)))

_NC_CACHE = {}


def _build_nc(repeats=1):
    if repeats in _NC_CACHE:
        return _NC_CACHE[repeats]
    nc = bacc.Bacc("TRN2", target_bir_lowering=False, debug=False,
                   num_devices=N_CORES)
    Q = nc.dram_tensor("Qb", [S, H], dt.float32, kind="ExternalInput").ap()
    K = nc.dram_tensor("Kb", [S, H], dt.float32, kind="ExternalInput").ap()
    V = nc.dram_tensor("Vb", [S, H], dt.float32, kind="ExternalInput").ap()
    WqT = nc.dram_tensor("WqT", [H, H], dt.float32, kind="ExternalInput").ap()
    WkT = nc.dram_tensor("WkT", [H, H], dt.float32, kind="ExternalInput").ap()
    WvT = nc.dram_tensor("WvT", [H, H], dt.float32, kind="ExternalInput").ap()
    WoT = nc.dram_tensor("WoT", [H, H], dt.float32, kind="ExternalInput").ap()
    Ob = nc.dram_tensor("Ob", [S, H], dt.float32, kind="ExternalOutput").ap()

    with tile.TileContext(nc) as tc:
        for _ in range(repeats):
            _emit(nc, tc, Q, K, V, WqT, WkT, WvT, WoT, Ob)
    nc.finalize()
    _NC_CACHE[repeats] = nc
    return nc


def _emit(nc, tc, Q, K, V, WqT, WkT, WvT, WoT, Ob):
    with (
        tc.tile_pool(name="const", bufs=1) as constp,
        tc.tile_pool(name="wstage", bufs=2) as wstage,
        tc.tile_pool(name="wr", bufs=2) as wr,
        tc.tile_pool(name="natstage", bufs=3) as natstage,
        tc.tile_pool(name="xtraw", bufs=1) as xtraw,
        tc.tile_pool(name="proj", bufs=1) as projp,
        tc.tile_pool(name="wtil", bufs=3) as wtil,
        tc.tile_pool(name="small", bufs=4) as smallp,
        tc.tile_pool(name="bcast", bufs=2) as bcastp,
        tc.tile_pool(name="outstage", bufs=2) as outstage,
    ):
        ident = constp.tile([128, 128], dt.float32, tag="ident")
        make_identity(nc, ident[:])

        qt = projp.tile([128, PAIRS * S], dt.float32r, tag="qt")
        kt = projp.tile([128, PAIRS * S], dt.float32r, tag="kt")
        v_aug = projp.tile([128, ST * VA], dt.float32r, tag="vaug")
        cT = projp.tile([128, PAIRS * S], dt.float32r, tag="ct")

        # ================= prep phase: transposes + projections ============
        with (
            tc.tile_pool(name="ps_ta", bufs=2, space="PSUM") as ps_ta,
            tc.tile_pool(name="ps_tb", bufs=2, space="PSUM") as ps_tb,
            tc.tile_pool(name="ps_p", bufs=3, space="PSUM") as ps_p,
        ):
            def load_weight(wdram):
                w = wr.tile([128, HT * H], dt.float32r, tag="w")
                for ht in range(HT):
                    stg = wstage.tile([128, H], dt.float32, tag="wstg")
                    nc.sync.dma_start(stg[:], wdram[ht * 128:(ht + 1) * 128, :])
                    nc.vector.tensor_copy(w[:, ht * H:(ht + 1) * H], stg[:])
                return w

            def transpose_in(xdram, dest):
                # dest: [128, HT*S] f32r ; dest[p, ht*S + s] = X[s, ht*128+p]
                destr = dest.rearrange("p (ht s) -> p ht s", ht=HT)
                for st in range(ST):
                    nat = natstage.tile([128, H], dt.float32, tag="nat")
                    nc.sync.dma_start(nat[:], xdram[st * 128:(st + 1) * 128, :])
                    pa = ps_ta.tile([128, 512], dt.float32, tag="ps_ta")
                    pb = ps_tb.tile([128, 256], dt.float32, tag="ps_tb")
                    for ht in range(HT):
                        pdst = (pa[:, ht * 128:(ht + 1) * 128] if ht < 4
                                else pb[:, (ht - 4) * 128:(ht - 3) * 128])
                        nc.tensor.transpose(
                            pdst, nat[:, ht * 128:(ht + 1) * 128], ident[:])
                    ss = slice(st * 128, (st + 1) * 128)
                    nc.vector.tensor_copy(
                        destr[:, 0:4, ss],
                        pa[:].rearrange("p (ht s) -> p ht s", ht=4))
                    nc.vector.tensor_copy(
                        destr[:, 4:6, ss],
                        pb[:].rearrange("p (ht s) -> p ht s", ht=2))

            def proj_os(w, xt, dest):
                # dest [128, PAIRS*S] f32r; dest[p, j*S+s] head-pair tile j
                for j in range(HT):
                    for b in range(S // 512):
                        ps = ps_p.tile([128, 512], dt.float32, tag="ps_p")
                        for ht in range(HT):
                            nc.tensor.matmul(
                                ps[:],
                                w[:, ht * H + j * 128: ht * H + (j + 1) * 128],
                                xt[:, ht * S + b * 512: ht * S + (b + 1) * 512],
                                start=(ht == 0), stop=(ht == HT - 1))
                        nc.scalar.copy(
                            dest[:, j * S + b * 512: j * S + (b + 1) * 512],
                            ps[:])

            def proj_v(w, vt, dest):
                for st in range(ST):
                    row = dest[:, st * VA:(st + 1) * VA]
                    rowr = row.rearrange("p (n d) -> p n d", d=DK + 1)
                    for o0, ow in ((0, 512), (512, 256)):
                        ps = ps_p.tile([128, 512], dt.float32, tag="ps_p")
                        for ht in range(HT):
                            nc.tensor.matmul(
                                ps[:, 0:ow],
                                vt[:, ht * S + st * 128: ht * S + (st + 1) * 128],
                                w[:, ht * H + o0: ht * H + o0 + ow],
                                start=(ht == 0), stop=(ht == HT - 1))
                        psr = ps[:, 0:ow].rearrange("p (n d) -> p n d", d=DK)
                        n0 = o0 // DK
                        nw = ow // DK
                        nc.vector.tensor_copy(rowr[:, n0:n0 + nw, 0:DK],
                                              psr[:])
                    nc.scalar.activation(
                        rowr[:, :, DK:DK + 1],
                        ident[:, 0:NH].rearrange("p (n o) -> p n o", o=1),
                        AF.Identity, bias=1.0, scale=0.0)

            qt_raw = xtraw.tile([128, HT * S], dt.float32r, tag="xt")
            transpose_in(Q, qt_raw)
            wq = load_weight(WqT)
            proj_os(wq, qt_raw, qt)

            kt_raw = xtraw.tile([128, HT * S], dt.float32r, tag="xt")
            transpose_in(K, kt_raw)
            wk = load_weight(WkT)
            proj_os(wk, kt_raw, kt)

            vt_raw = xtraw.tile([128, HT * S], dt.float32r, tag="xt")
            transpose_in(V, vt_raw)
            wv = load_weight(WvT)
            proj_v(wv, vt_raw, v_aug)

            wo = load_weight(WoT)

        # ================= attention + output projection ===================
        var = v_aug[:, :].rearrange("p (st n d) -> p st n d", st=ST, d=DK + 1)
        with (
            tc.tile_pool(name="ps_s", bufs=4, space="PSUM") as ps_s,
            tc.tile_pool(name="ps_c", bufs=2, space="PSUM") as ps_c,
            tc.tile_pool(name="ps_o", bufs=2, space="PSUM") as ps_o,
        ):
            for j in range(PAIRS):
                for b in range(SQB):
                    sq = slice(j * S + b * 512, j * S + (b + 1) * 512)
                    pc = []
                    for h in range(2):
                        pch = ps_c.tile([DK + 1, 512], dt.float32, tag="ps_c")
                        pc.append(pch)
                    for t in range(ST):
                        sk = slice(j * S + t * 128, j * S + (t + 1) * 128)
                        for h in range(2):
                            p0 = h * DK
                            pss = ps_s.tile([128, 512], dt.float32, tag="ps_s")
                            nc.tensor.matmul(
                                pss[:],
                                kt[p0:p0 + DK, sk], qt[p0:p0 + DK, sq],
                                start=True, stop=True, tile_position=(p0, 0))
                            w_t = wtil.tile([128, 512], dt.float32r, tag="wt")
                            nc.scalar.activation(w_t[:], pss[:], AF.Exp,
                                                 scale=SCALE)
                            nc.tensor.matmul(
                                pc[h][:], var[:, t, 2 * j + h, :], w_t[:],
                                start=(t == 0), stop=(t == ST - 1),
                                skip_group_check=True)
                    for h in range(2):
                        rec = smallp.tile([1, 512], dt.float32, tag="rec")
                        nc.vector.reciprocal(rec[:], pc[h][DK:DK + 1, :])
                        bc = bcastp.tile([DK, 512], dt.float32, tag="bc")
                        nc.gpsimd.partition_broadcast(bc[:], rec[:])
                        nc.vector.tensor_mul(
                            cT[h * DK:(h + 1) * DK, sq],
                            pc[h][0:DK, :], bc[:])

            for st in range(ST):
                out_sb = outstage.tile([128, H], dt.float32, tag="out")
                for o0, ow in ((0, 512), (512, 256)):
                    ps = ps_o.tile([128, 512], dt.float32, tag="ps_o")
                    for j in range(PAIRS):
                        nc.tensor.matmul(
                            ps[:, 0:ow],
                            cT[:, j * S + st * 128: j * S + (st + 1) * 128],
                            wo[:, j * H + o0: j * H + o0 + ow],
                            start=(j == 0), stop=(j == PAIRS - 1))
                    nc.scalar.copy(out_sb[:, o0:o0 + ow], ps[:, 0:ow])
                nc.sync.dma_start(Ob[st * 128:(st + 1) * 128, :], out_sb[:])


def kernel(Q, K, V, Wq, Wk, Wv, Wo):
    nc = _build_nc()
    wqt = np.ascontiguousarray(np.asarray(Wq).T)
    wkt = np.ascontiguousarray(np.asarray(Wk).T)
    wvt = np.ascontiguousarray(np.asarray(Wv).T)
    wot = np.ascontiguousarray(np.asarray(Wo).T)
    in_maps = []
    for c in range(N_CORES):
        in_maps.append({
            "Qb": np.ascontiguousarray(np.asarray(Q)[c]),
            "Kb": np.ascontiguousarray(np.asarray(K)[c]),
            "Vb": np.ascontiguousarray(np.asarray(V)[c]),
            "WqT": wqt, "WkT": wkt, "WvT": wvt, "WoT": wot,
        })
    res = bass_utils.run_bass_kernel_spmd(nc, in_maps,
                                          core_ids=list(range(N_CORES)))
    out = np.stack([res.results[c]["Ob"] for c in range(N_CORES)], axis=0)
    return out.astype(np.float32)
